# revision 3
# baseline (speedup 1.0000x reference)
"""Contrastive-loss (InfoNCE re-rank) Trainium2 Bass kernel.

Full op: q,k [256,1024], neg [256,2048,1024] f32.
  l_pos[n]   = q[n].k[n]
  l_neg[n,j] = neg[n,j].q[n]
  loss = mean_n( LSE(logits_n/T) - l_pos[n]/T ),  T = 0.07

Sharding: data-parallel over N across 8 NeuronCores (32 samples/core);
each core computes its 32 per-sample NLLs, host takes the mean.

Layout: since K = 2048 = 4*512, the per-core neg shard [32, 2048, 1024]
reshapes CONTIGUOUSLY to [128, 512, 1024]: partition p <-> (sample
n=p//4, negative-group g=p%4). Each chunk DMA is then a single 2-D
[128, 16*1024] transfer reading one contiguous 64 KB run per partition
(a 3-D [P, jc, C] tile lowers to jc sub-DMAs that each pay ~2us
completion latency — 2x slower end to end). The q broadcast (each
partition needs q[p//4]) is done host-side by passing qb =
repeat(q, 4, axis=0) as an extra 512 KB input: one plain DMA, no
SBUF->SBUF traffic. DVE does one fused multiply+reduce
(scalar_tensor_tensor, scale folded in) per [128, 1024] slab; the
32-chunk DMA stream double-buffers ahead of it. LSE is two-level:
free-dim LSE over each partition's 512 logits, then a 128->32x4
cross-partition DMA reshuffle + 5-way combine with the positive logit.

Measured (repeat-slope, amortized over in-kernel re-streams): ~1.23 ms
per pass vs ~5.7 ms for the per-sample baseline. All-core aggregate
~1.75 TB/s HBM read — this platform's wall for simultaneous 8-core
streaming (single-core runs no faster per core).
"""

import numpy as np

import concourse.bass as bass
import concourse.bacc as bacc
import concourse.tile as tile
from concourse import mybir
from concourse.bass_utils import run_bass_kernel_spmd

N, C, K = 256, 1024, 2048
NCORES = 8
NLOC = N // NCORES          # 32 samples per core
P = 128                     # SBUF partitions
G = P // NLOC               # 4 negative-groups per sample
JG = K // G                 # 512 negatives per (sample, group) partition
JC = 16                     # negatives per chunk DMA (8 MB per dma_start)
NEG_BUFS = 2                # chunk buffers (NEG_BUFS*JC*4KB per partition)
TEMP = 0.07
SCALE = 1.0 / TEMP
F32 = mybir.dt.float32
ALU = mybir.AluOpType
ACT = mybir.ActivationFunctionType


def build_module(repeat: int = 1, jc: int = None, bufs: int = None) -> bass.Bass:
    """repeat>1 re-streams the whole shard (benchmark mode: the
    steady-state per-pass time is the kernel's true throughput; the
    logits are just overwritten, so the result is unchanged)."""
    jc = JC if jc is None else jc
    bufs = NEG_BUFS if bufs is None else bufs
    nch = JG // jc

    # Bacc (not plain Bass): its compile() runs generate_event_semaphores,
    # which splits multi-sem waits into separate event instructions — this
    # walrus rejects >1 sync wait per instruction.
    nc = bacc.Bacc("TRN2", target_bir_lowering=False)
    q_d = nc.dram_tensor("q", [NLOC, C], F32, kind="ExternalInput")
    k_d = nc.dram_tensor("k", [NLOC, C], F32, kind="ExternalInput")
    qb_d = nc.dram_tensor("qb", [P, C], F32, kind="ExternalInput")
    neg_d = nc.dram_tensor("neg", [P, JG, C], F32, kind="ExternalInput")
    out_d = nc.dram_tensor("nll", [NLOC, 1], F32, kind="ExternalOutput")

    with tile.TileContext(nc) as tc:
        with (
            tc.tile_pool(name="consts", bufs=1) as consts,
            tc.tile_pool(name="small", bufs=1) as small,
            tc.tile_pool(name="negp", bufs=bufs) as negp,
        ):
            qb_sb = consts.tile([P, C], F32)
            nc.sync.dma_start(out=qb_sb, in_=qb_d[:])
            q_sb = consts.tile([NLOC, C], F32)
            k_sb = consts.tile([NLOC, C], F32)
            nc.sync.dma_start(out=q_sb, in_=q_d[:])
            nc.sync.dma_start(out=k_sb, in_=k_d[:])

            # y_pos[n] = (q[n].k[n]) / T, one fused mult+reduce.
            pos_scr = small.tile([NLOC, C], F32)
            ypos = small.tile([NLOC, 1], F32)
            nc.vector.scalar_tensor_tensor(
                out=pos_scr, in0=q_sb, scalar=SCALE, in1=k_sb,
                op0=ALU.mult, op1=ALU.mult, accum_out=ypos,
            )

            # Scaled negative logits Y[p, j] = neg[p, j].qb[p] / T.
            Y = small.tile([P, JG], F32)
            scr = small.tile([P, C], F32)  # discarded elementwise product
            neg_ap = neg_d[:]
            for _r in range(repeat):
                for i in range(nch):
                    t = negp.tile([P, jc * C], F32)
                    src = bass.AP(
                        tensor=neg_ap.tensor,
                        offset=neg_ap.offset + i * jc * C,
                        ap=[[JG * C, P], [1, jc * C]],
                    )
                    eng = nc.sync if i % 2 == 0 else nc.scalar
                    eng.dma_start(out=t, in_=src)
                    for j in range(jc):
                        jj = i * jc + j
                        nc.vector.scalar_tensor_tensor(
                            out=scr, in0=t[:, j * C : (j + 1) * C],
                            scalar=SCALE, in1=qb_sb,
                            op0=ALU.mult, op1=ALU.mult,
                            accum_out=Y[:, jj : jj + 1],
                        )

            # Level-1 LSE along the free dim: per (n, g) partition over 512.
            m1 = small.tile([P, 1], F32)
            nc.vector.reduce_max(out=m1, in_=Y, axis=mybir.AxisListType.X)
            m1n = small.tile([P, 1], F32)
            nc.scalar.mul(m1n, m1, -1.0)
            e_scr = small.tile([P, JG], F32)
            s1 = small.tile([P, 1], F32)
            nc.scalar.activation(
                out=e_scr, in_=Y, func=ACT.Exp,
                bias=m1n, scale=1.0, accum_out=s1,
            )
            lse_p = small.tile([P, 1], F32)
            nc.scalar.activation(out=lse_p, in_=s1, func=ACT.Ln)
            nc.vector.tensor_add(out=lse_p, in0=lse_p, in1=m1)

            # Level-2: gather each sample's 4 group-LSEs into the free dim
            # (cross-partition DMA reshuffle), append y_pos as 5th column.
            lt = small.tile([NLOC, G + 1], F32)
            nc.sync.dma_start(out=lt[:, 0:G], in_=lse_p[:])
            nc.vector.tensor_copy(out=lt[:, G : G + 1], in_=ypos)

            m2 = small.tile([NLOC, 1], F32)
            nc.vector.reduce_max(out=m2, in_=lt, axis=mybir.AxisListType.X)
            m2n = small.tile([NLOC, 1], F32)
            nc.scalar.mul(m2n, m2, -1.0)
            e2 = small.tile([NLOC, G + 1], F32)
            s2 = small.tile([NLOC, 1], F32)
            nc.scalar.activation(
                out=e2, in_=lt, func=ACT.Exp,
                bias=m2n, scale=1.0, accum_out=s2,
            )
            ln2 = small.tile([NLOC, 1], F32)
            nc.scalar.activation(out=ln2, in_=s2, func=ACT.Ln)

            # nll[n] = (m2 + ln s2) - y_pos[n]
            nll = small.tile([NLOC, 1], F32)
            nc.vector.tensor_scalar(
                out=nll, in0=ln2, scalar1=m2, scalar2=ypos,
                op0=ALU.add, op1=ALU.subtract,
            )
            nc.sync.dma_start(out=out_d[:], in_=nll)

    nc.finalize()
    return nc


_CACHED = {}


def make_in_maps(q, k, neg):
    in_maps = []
    for c in range(NCORES):
        s = slice(c * NLOC, (c + 1) * NLOC)
        qs = np.ascontiguousarray(q[s])
        in_maps.append({
            "q": qs,
            "k": np.ascontiguousarray(k[s]),
            "qb": np.repeat(qs, G, axis=0),
            "neg": neg[s].reshape(P, JG, C),
        })
    return in_maps


def finalize_output(per_core):
    """per_core: {"nll": [NCORES, NLOC, 1]} -> scalar f32 loss."""
    return np.asarray(
        np.mean(per_core["nll"].astype(np.float64)), dtype=np.float32
    )


def _run(q, k, neg, trace=False):
    if "nc" not in _CACHED:
        _CACHED["nc"] = build_module()
    nc = _CACHED["nc"]
    in_maps = make_in_maps(q, k, neg)
    res = run_bass_kernel_spmd(
        nc, in_maps, core_ids=list(range(NCORES)), trace=trace
    )
    nll = np.stack([r["nll"] for r in res.results])
    loss = finalize_output({"nll": nll})
    return loss, res


def kernel(q, k, neg):
    q = np.ascontiguousarray(np.asarray(q, dtype=np.float32))
    k = np.ascontiguousarray(np.asarray(k, dtype=np.float32))
    neg = np.ascontiguousarray(np.asarray(neg, dtype=np.float32))
    loss, _ = _run(q, k, neg, trace=False)
    return loss
